# revision 1
# baseline (speedup 1.0000x reference)
"""Trainium2 Bass kernel for nn_AutoregressiveMixerBlock.

Reference computation (per batch b):
  y  = LN_H(x)                                    # layer norm over H
  t  = revcumsum_N(y)                             # t[j] = sum_{i>=j} y[i]
  h  = gelu(t^T @ tok_w1 + tok_b1)                # [H, TM]
  y2 = (h @ tok_w2 + tok_b2)^T                    # [N, H]
  y3 = LN_H(y2)
  out = gelu(y3 @ ch_w1 + ch_b1) @ ch_w2 + ch_b2  # [N, H]

Key algebraic folds (exact in real arithmetic, applied on host):
  * revcumsum+matmul:  sum_j t[j,h] w1[j,m] = sum_i y[i,h] W1c[i,m]
    with W1c = cumsum(tok_w1, axis=0) -> no on-device cumsum at all.
  * LN1 gain/bias move past the token matmul:
    out1[h,m] = g[h] * (yn^T @ W1c)[h,m] + (b[h]*colsum1[m] + tok_b1[m])
  * tok_b2 and the LN2 mean both vanish by centering h^T by its
    per-row (over H) mean before the second token matmul.
  * LN2 gain/bias fold into ch_w1 / ch_b1.

Sharding: data-parallel over B across 8 cores (2 batches per core),
weights replicated.
"""

import numpy as np

B, N, H = 16, 8192, 128
TM, CM = 256, 512
EPS = 1e-5
NCORES = 8
BL = B // NCORES          # batches per core
P = 128                   # partitions
NC_TOK = N // P           # 64 token chunks of 128
NJ = N // 512             # 16 column chunks of 512
KTM = TM // P             # 2 k-chunks for the second token matmul
NCI = CM // P             # 4 chunks of the channel hidden dim

_cached = {}


def _build(nontrivial_bias1, nontrivial_cb2):
    import concourse.bass as bass
    import concourse.mybir as mybir
    import concourse.tile as tile
    from concourse import bacc
    from concourse.masks import make_identity
    import bass_rust

    F32 = mybir.dt.float32
    F32R = mybir.dt.float32r
    BF16 = mybir.dt.bfloat16
    AF = mybir.ActivationFunctionType
    ALU = mybir.AluOpType
    AX = mybir.AxisListType

    nc = bacc.Bacc()

    # ---- DRAM tensors -------------------------------------------------
    x_d = nc.dram_tensor("x", [BL, N, H], F32, kind="ExternalInput")
    w1c_d = nc.dram_tensor("w1c", [N, TM], F32R, kind="ExternalInput")
    w2_d = nc.dram_tensor("w2", [TM, N], F32R, kind="ExternalInput")
    g1_d = nc.dram_tensor("g1", [P, 1], F32, kind="ExternalInput")
    bias1_d = nc.dram_tensor("bias1", [P, TM], F32, kind="ExternalInput")
    cw1_d = nc.dram_tensor("cw1", [H, CM], BF16, kind="ExternalInput")
    cb1_d = nc.dram_tensor("cb1", [P, NCI], F32, kind="ExternalInput")
    cw2_d = nc.dram_tensor("cw2", [CM, H], BF16, kind="ExternalInput")
    cb2_d = nc.dram_tensor("cb2", [P, 1], F32, kind="ExternalInput")
    ones_d = nc.dram_tensor("ones", [P, P], F32R, kind="ExternalInput")
    out_d = nc.dram_tensor("out", [BL, H, N], F32, kind="ExternalOutput")

    # DRAM views
    x_v = [x_d[b].rearrange("(c p) h -> p c h", p=P) for b in range(BL)]
    w1c_v = w1c_d[:].rearrange("(c p) m -> p c m", p=P)
    w2_v = w2_d[:].rearrange("(k p) (j n) -> p k j n", p=P, n=512)
    cw2_v = cw2_d[:].rearrange("(ci p) h -> p ci h", p=P)
    out_v = [out_d[b] for b in range(BL)]

    act_phases = [[], [], [], []]  # ACT table-set phase buckets

    with tile.TileContext(nc) as tc:
        import contextlib
        with contextlib.ExitStack() as ctx:
            const = ctx.enter_context(tc.tile_pool(name="const", bufs=1))
            xall = ctx.enter_context(tc.tile_pool(name="xall", bufs=BL))
            stats = ctx.enter_context(tc.tile_pool(name="stats", bufs=2 * BL))
            small = ctx.enter_context(tc.tile_pool(name="small", bufs=4))
            sqp = ctx.enter_context(tc.tile_pool(name="sqp", bufs=1))
            w1cs = ctx.enter_context(tc.tile_pool(name="w1cs", bufs=4))
            w2s = ctx.enter_context(tc.tile_pool(name="w2s", bufs=6))
            sq2p = ctx.enter_context(tc.tile_pool(name="sq2p", bufs=3))
            rstdp = ctx.enter_context(tc.tile_pool(name="rstdp", bufs=3))
            g2p = ctx.enter_context(tc.tile_pool(name="g2p", bufs=2))
            outp = ctx.enter_context(tc.tile_pool(name="outp", bufs=3))

            # ---- constants -------------------------------------------
            g1_sb = const.tile([P, 1], F32)
            nc.sync.dma_start(g1_sb, g1_d[:])
            cw1_sb = const.tile([H, CM], BF16)
            nc.sync.dma_start(cw1_sb, cw1_d[:])
            cb1_sb = const.tile([P, NCI], F32)
            nc.sync.dma_start(cb1_sb, cb1_d[:])
            cw2_sb = const.tile([P, NCI, H], BF16)
            nc.sync.dma_start(cw2_sb, cw2_v)
            ones_sb = const.tile([P, P], F32R)
            nc.sync.dma_start(ones_sb, ones_d[:])
            ident = const.tile([P, P], F32)
            make_identity(nc, ident)
            if nontrivial_bias1:
                bias1_sb = const.tile([P, TM], F32)
                nc.sync.dma_start(bias1_sb, bias1_d[:])
            if nontrivial_cb2:
                cb2_sb = const.tile([P, 1], F32)
                nc.sync.dma_start(cb2_sb, cb2_d[:])
                cb2_t = small.tile([P, 1], F32, tag="cb2t")
                nc.vector.tensor_copy(cb2_t, cb2_sb)
            # pre-touch the per-partition scalar so later scalar-pointer
            # ops don't need a DMA wait of their own
            g1_t = small.tile([P, 1], F32)
            nc.vector.tensor_copy(g1_t, g1_sb)
            eps_t = const.tile([P, 1], F32)
            nc.vector.memset(eps_t, EPS)

            # ---- phase 1: LN1 stats + normalize + token matmul 1 -----
            x_sb = []
            rstd1 = []
            mu1 = []
            for b in range(BL):
                xt = xall.tile([P, NC_TOK, H], F32, tag="xall", name=f"xall{b}")
                nc.sync.dma_start(xt, x_v[b])
                x_sb.append(xt)

                sums = stats.tile([P, NC_TOK], F32, tag="st_sum")
                nc.vector.tensor_reduce(
                    out=sums, in_=xt, axis=AX.X, op=ALU.add)
                sq = sqp.tile([P, NC_TOK, H], BF16, tag="sq")
                i_sq = nc.scalar.activation(sq, xt, AF.Square)
                act_phases[0].append(i_sq)
                sumsq = stats.tile([P, NC_TOK], F32, tag="st_sumsq")
                nc.vector.tensor_reduce(
                    out=sumsq, in_=sq, axis=AX.X, op=ALU.add)

                mu = stats.tile([P, NC_TOK], F32, tag="st_mu")
                nc.vector.tensor_scalar_mul(mu, sums, 1.0 / H)
                ex2 = stats.tile([P, NC_TOK], F32, tag="st_ex2")
                nc.vector.tensor_scalar_mul(ex2, sumsq, 1.0 / H)
                musq = stats.tile([P, NC_TOK], F32, tag="st_musq")
                nc.vector.tensor_tensor(musq, mu, mu, ALU.mult)
                var = stats.tile([P, NC_TOK], F32, tag="st_var")
                nc.vector.tensor_tensor(var, ex2, musq, ALU.subtract)
                nc.vector.tensor_scalar(
                    out=var, in0=var, scalar1=EPS, scalar2=None, op0=ALU.add)
                std = stats.tile([P, NC_TOK], F32, tag="st_std")
                i_r = nc.scalar.activation(std, var, AF.Sqrt)
                act_phases[0].append(i_r)
                rst = stats.tile([P, NC_TOK], F32, tag="st_rstd")
                nc.vector.reciprocal_approx_fast(rst, std)
                rstd1.append(rst)
                mu1.append(mu)

            with (
                tc.tile_pool(name="ps1", bufs=BL, space="PSUM") as ps1,
                tc.tile_pool(name="pst", bufs=2, space="PSUM") as pst,
            ):
                psum1 = [ps1.tile([P, TM], F32, tag="ps1", name=f"ps1_{b}")
                         for b in range(BL)]
                for c in range(NC_TOK):
                    w1t = w1cs.tile([P, TM], F32R, tag="w1c")
                    nc.sync.dma_start(w1t, w1c_v[:, c, :])
                    for b in range(BL):
                        xn = small.tile([P, P], F32R, tag="xn")
                        nc.vector.tensor_scalar(
                            out=xn,
                            in0=x_sb[b][:, c, :],
                            scalar1=mu1[b][:, c:c + 1],
                            scalar2=rstd1[b][:, c:c + 1],
                            op0=ALU.subtract,
                            op1=ALU.mult,
                        )
                        nc.tensor.matmul(
                            psum1[b],
                            xn,
                            w1t,
                            start=(c == 0),
                            stop=(c == NC_TOK - 1),
                        )

                # ---- phase 2: token gelu, transpose, center --------------
                h1c = []  # per batch: list of KTM [P, P] f32r tiles
                for b in range(BL):
                    h1 = small.tile([P, TM], F32, tag="h1")
                    if nontrivial_bias1:
                        nc.vector.tensor_scalar_mul(h1, psum1[b], g1_t)
                        nc.vector.tensor_add(h1, h1, bias1_sb)
                        i_g = nc.scalar.activation(h1, h1, AF.Gelu)
                    else:
                        i_g = nc.scalar.activation(h1, psum1[b], AF.Gelu,
                                                   scale=g1_t)
                    act_phases[1].append(i_g)

                    chunks = []
                    for k in range(KTM):
                        ps_t = pst.tile([P, P], F32, tag="pst")
                        nc.tensor.transpose(ps_t, h1[:, k * P:(k + 1) * P], ident)
                        h1T = small.tile([P, P], F32, tag="h1T")
                        nc.vector.tensor_copy(h1T, ps_t)
                        hsum = small.tile([P, 1], F32, tag="hsum")
                        nc.vector.tensor_reduce(
                            out=hsum, in_=h1T, axis=AX.X, op=ALU.add)
                        hmean = small.tile([P, 1], F32, tag="hmean")
                        nc.vector.tensor_scalar_mul(hmean, hsum, 1.0 / H)
                        hc = small.tile([P, P], F32R, tag="h1c")
                        nc.vector.tensor_scalar(
                            out=hc, in0=h1T, scalar1=hmean, scalar2=None,
                            op0=ALU.subtract)
                        chunks.append(hc)
                    h1c.append(chunks)

            # ---- phase 3a: token matmul 2 + LN2 stats ----------------
            with (
                tc.tile_pool(name="ps2", bufs=3, space="PSUM") as ps2,
                tc.tile_pool(name="psv", bufs=2, space="PSUM") as psv,
            ):
                y2n = []
                for b in range(BL):
                    y2n.append(xall.tile([P, N], BF16, tag="xall", name=f"y2n{b}"))

                for j in range(NJ):
                    w2t = []
                    for k in range(KTM):
                        wt = w2s.tile([P, 512], F32R, tag="w2")
                        nc.sync.dma_start(wt, w2_v[:, k, j, :])
                        w2t.append(wt)
                    for b in range(BL):
                        p2 = ps2.tile([P, 512], F32, tag="ps2")
                        for k in range(KTM):
                            nc.tensor.matmul(
                                p2, h1c[b][k], w2t[k],
                                start=(k == 0), stop=(k == KTM - 1))
                        sq2 = sq2p.tile([P, 512], F32R, tag="sq2")
                        i_s = nc.scalar.activation(
                            sq2, p2, AF.Square, scale=float(1.0 / np.sqrt(H)))
                        act_phases[2].append(i_s)
                        vps = psv.tile([P, 512], F32, tag="psv")
                        nc.tensor.matmul(vps, ones_sb, sq2, start=True, stop=True)
                        std = rstdp.tile([P, 512], F32, tag="std")
                        i_r = nc.scalar.activation(std, vps, AF.Sqrt, bias=eps_t)
                        act_phases[2].append(i_r)
                        rstd = rstdp.tile([P, 512], F32, tag="rstd")
                        nc.vector.reciprocal_approx_fast(rstd, std)
                        nc.vector.tensor_tensor(
                            y2n[b][:, j * 512:(j + 1) * 512],
                            p2, rstd, ALU.mult)

            # ---- phase 3b: channel MLP -------------------------------
            with (
                tc.tile_pool(name="psr", bufs=1, space="PSUM") as psr,
                tc.tile_pool(name="pso", bufs=2, space="PSUM") as pso,
            ):
                for j in range(NJ):
                    for b in range(BL):
                        y2s = y2n[b][:, j * 512:(j + 1) * 512]
                        raw2 = psr.tile([P, NCI * 512], F32, tag="psr")
                        for ci in range(NCI):
                            nc.tensor.matmul(
                                raw2[:, ci * 512:(ci + 1) * 512],
                                cw1_sb[:, ci * P:(ci + 1) * P],
                                y2s, start=True, stop=True)
                        g2 = g2p.tile([P, NCI * 512], BF16, tag="g2")
                        if nontrivial_bias1:
                            # general path: per-ci bias
                            for ci in range(NCI):
                                i_g = nc.scalar.activation(
                                    g2[:, ci * 512:(ci + 1) * 512],
                                    raw2[:, ci * 512:(ci + 1) * 512],
                                    AF.Gelu, bias=cb1_sb[:, ci:ci + 1])
                                act_phases[3].append(i_g)
                        else:
                            i_g = nc.scalar.activation(g2, raw2, AF.Gelu)
                            act_phases[3].append(i_g)

                        po = pso.tile([P, 512], F32, tag="pso")
                        for ci in range(NCI):
                            nc.tensor.matmul(
                                po,
                                cw2_sb[:, ci, :],
                                g2[:, ci * 512:(ci + 1) * 512],
                                start=(ci == 0), stop=(ci == NCI - 1))
                        osb = outp.tile([P, 512], F32, tag="osb")
                        if nontrivial_cb2:
                            nc.vector.tensor_scalar(
                                out=osb, in0=po, scalar1=cb2_t, scalar2=None,
                                op0=ALU.add)
                        else:
                            nc.vector.tensor_copy(osb, po)
                        nc.sync.dma_start(
                            out_v[b][:, j * 512:(j + 1) * 512], osb)

            # ---- ACT table-set ordering edges ------------------------
            for ph in range(3):
                for f in act_phases[ph + 1]:
                    for t in act_phases[ph]:
                        bass_rust.add_dep_helper(
                            f.ins, t.ins, sync=False,
                            reason="act table set phase ordering")

    nc.compile()
    return nc


def _host_prep(inputs):
    x = np.ascontiguousarray(inputs["x"], dtype=np.float32)
    ln1_g = np.asarray(inputs["ln1_g"], np.float32)
    ln1_b = np.asarray(inputs["ln1_b"], np.float32)
    ln2_g = np.asarray(inputs["ln2_g"], np.float32)
    ln2_b = np.asarray(inputs["ln2_b"], np.float32)
    tok_w1 = np.asarray(inputs["tok_w1"], np.float32)
    tok_b1 = np.asarray(inputs["tok_b1"], np.float32)
    tok_w2 = np.asarray(inputs["tok_w2"], np.float32)
    ch_w1 = np.asarray(inputs["ch_w1"], np.float32)
    ch_b1 = np.asarray(inputs["ch_b1"], np.float32)
    ch_w2 = np.asarray(inputs["ch_w2"], np.float32)
    ch_b2 = np.asarray(inputs["ch_b2"], np.float32)

    import ml_dtypes
    w1c = np.cumsum(tok_w1, axis=0, dtype=np.float64).astype(np.float32)
    colsum1 = w1c.sum(axis=0, dtype=np.float64).astype(np.float32)
    bias1 = ln1_b[:, None] * colsum1[None, :] + tok_b1[None, :]
    cw1 = (ln2_g[:, None] * ch_w1).astype(np.float32)
    cb1 = (ch_b1 + ch_w1.T @ ln2_b).astype(np.float32)
    cw2 = ch_w2.astype(ml_dtypes.bfloat16)


    nontrivial_bias1 = bool(np.any(bias1 != 0.0) or np.any(cb1 != 0.0))
    nontrivial_cb2 = bool(np.any(ch_b2 != 0.0))

    shared = {
        "w1c": w1c,
        "w2": np.ascontiguousarray(tok_w2),
        "g1": ln1_g.reshape(P, 1).copy(),
        "bias1": np.ascontiguousarray(bias1, np.float32),
        "cw1": cw1.astype(ml_dtypes.bfloat16),
        "cb1": np.ascontiguousarray(cb1.reshape(NCI, P).T.copy()),
        "cw2": np.ascontiguousarray(cw2),
        "cb2": ch_b2.reshape(P, 1).astype(np.float32).copy(),
        "ones": np.ones((P, P), np.float32),
    }
    return x, shared, nontrivial_bias1, nontrivial_cb2


def kernel(**inputs) -> np.ndarray:
    from concourse.bass_utils import run_bass_kernel_spmd

    x, shared, nb1, nb2 = _host_prep(inputs)

    key = (nb1, nb2)
    if key not in _cached:
        _cached[key] = _build(nb1, nb2)
    nc = _cached[key]

    in_maps = []
    for c in range(NCORES):
        m = dict(shared)
        m["x"] = np.ascontiguousarray(x[c * BL:(c + 1) * BL])
        in_maps.append(m)

    res = run_bass_kernel_spmd(nc, in_maps, core_ids=list(range(NCORES)))
    out = np.concatenate(
        [r["out"].transpose(0, 2, 1) for r in res.results], axis=0)
    return np.ascontiguousarray(out, dtype=np.float32)


if __name__ == "__main__":
    rng = np.random.default_rng(0)
    ins = {
        "x": rng.standard_normal((B, N, H)).astype(np.float32),
        "ln1_g": np.ones(H, np.float32),
        "ln1_b": np.zeros(H, np.float32),
        "ln2_g": np.ones(H, np.float32),
        "ln2_b": np.zeros(H, np.float32),
        "tok_w1": (rng.standard_normal((N, TM)) * 0.02).astype(np.float32),
        "tok_b1": np.zeros(TM, np.float32),
        "tok_w2": (rng.standard_normal((TM, N)) * 0.02).astype(np.float32),
        "tok_b2": np.zeros(N, np.float32),
        "ch_w1": (rng.standard_normal((H, CM)) * 0.02).astype(np.float32),
        "ch_b1": np.zeros(CM, np.float32),
        "ch_w2": (rng.standard_normal((CM, H)) * 0.02).astype(np.float32),
        "ch_b2": np.zeros(H, np.float32),
    }
    out = kernel(**ins)
    print("out", out.shape, out.dtype)



# revision 13
# speedup vs baseline: 1.2977x; 1.2977x over previous
"""Trainium2 Bass kernel for nn_AutoregressiveMixerBlock.

Reference computation (per batch b):
  y  = LN_H(x)                                    # layer norm over H
  t  = revcumsum_N(y)                             # t[j] = sum_{i>=j} y[i]
  h  = gelu(t^T @ tok_w1 + tok_b1)                # [H, TM]
  y2 = (h @ tok_w2 + tok_b2)^T                    # [N, H]
  y3 = LN_H(y2)
  out = gelu(y3 @ ch_w1 + ch_b1) @ ch_w2 + ch_b2  # [N, H]

Algebraic folds (exact in real arithmetic, applied on host):
  * revcumsum+matmul:  sum_j t[j,h] w1[j,m] = sum_i y[i,h] W1c[i,m]
    with W1c = cumsum(tok_w1, axis=0) -> no on-device cumsum at all.
  * LN1 gain/bias move past the token matmul.
  * tok_b2 and the LN2 mean both vanish by centering h^T by its
    per-row (over H) mean before the second token matmul.
  * LN2 gain/bias fold into ch_w1 / ch_b1.

Device strategy (per core, 2 batches):
  * all matmul operands bf16 (PE 1 cyc/row), f32 PSUM accumulate.
  * LN stats via bn_stats on DVE; rsqrt via int bit-hack + 2 Newton
    iterations on GPSIMD (no ACT Sqrt -> single ACT table set for the
    whole kernel: scalar engine only runs Gelu/Copy/Identity).
  * LN2 variance reduced to compact [token,1] psum columns with tiny
    ones-column matmuls; per-token rstd goes through a DRAM roundtrip
    (plain store, transposed load, store, stride-0 broadcast load)
    to become a [128, N] bf16 multiplier tile.
  * phases are software-pipelined across the two batches; the final
    channel-MLP accumulator reuses the first bank of the gelu-input
    psum tile so everything fits in 8 PSUM banks.
"""

import numpy as np

B, N, H = 16, 8192, 128
TM, CM = 256, 512
EPS = 1e-5
NCORES = 8
BL = B // NCORES          # batches per core
P = 128                   # partitions
NC_TOK = N // P           # 64 token chunks of 128
NJ = 16                   # j chunks per batch
JW = N // NJ              # 512 tokens per j chunk
TPJ = JW // P             # 4 token chunks per j chunk
KTM = TM // P             # 2 k-chunks for the second token matmul
NCI = CM // P             # 4 chunks of the channel hidden dim
NG = 16                   # bn_stats groups (4 chunks each)
GC = NC_TOK // NG         # chunks per group
MAGIC1 = 0x5F3759DF + 1   # fast-rsqrt magic (for the xor/asr variant)

_cached = {}


def _build(nontrivial_bias1, nontrivial_cb2):
    import contextlib

    import concourse.mybir as mybir
    import concourse.tile as tile
    from concourse import bacc
    from concourse.masks import make_identity

    F32 = mybir.dt.float32
    BF16 = mybir.dt.bfloat16
    I32 = mybir.dt.int32
    AF = mybir.ActivationFunctionType
    ALU = mybir.AluOpType
    AX = mybir.AxisListType

    nc = bacc.Bacc()

    # ---- DRAM tensors -------------------------------------------------
    x_d = nc.dram_tensor("x", [BL, N, H], BF16, kind="ExternalInput")
    w1c_d = nc.dram_tensor("w1c", [N, TM], BF16, kind="ExternalInput")
    w2_d = nc.dram_tensor("w2", [TM, N], BF16, kind="ExternalInput")
    g1_d = nc.dram_tensor("g1", [P, 1], F32, kind="ExternalInput")
    bias1_d = nc.dram_tensor("bias1", [P, TM], F32, kind="ExternalInput")
    cw1_d = nc.dram_tensor("cw1", [H, CM], BF16, kind="ExternalInput")
    cb1_d = nc.dram_tensor("cb1", [P, NCI], F32, kind="ExternalInput")
    cw2_d = nc.dram_tensor("cw2", [CM, H], BF16, kind="ExternalInput")
    cb2_d = nc.dram_tensor("cb2", [P, 1], F32, kind="ExternalInput")
    onesc_d = nc.dram_tensor("onesc", [P, 1], BF16, kind="ExternalInput")
    # rstd scratch: [t, c] then [c, t] layouts (DMA-transposed between)
    rs1_d = nc.dram_tensor("rs1", [BL, P, P], BF16,
                           kind="ExternalOutput")
    rs2_d = nc.dram_tensor("rs2", [BL, NC_TOK, P], BF16,
                           kind="ExternalOutput")
    out_d = nc.dram_tensor("out", [BL, H, N], F32, kind="ExternalOutput")

    # DRAM views
    x_v = [x_d[b].rearrange("(c p) h -> p c h", p=P) for b in range(BL)]
    w1c_v = w1c_d[:].rearrange("(c p) m -> p c m", p=P)
    w2_v = w2_d[:].rearrange("(k p) (j n) -> p k j n", p=P, n=JW)
    cw2_v = cw2_d[:].rearrange("(ci p) h -> p ci h", p=P)
    out_v = [out_d[b] for b in range(BL)]

    with tile.TileContext(nc) as tc:
        with contextlib.ExitStack() as ctx:
            const = ctx.enter_context(tc.tile_pool(name="const", bufs=1))
            big = ctx.enter_context(tc.tile_pool(name="big", bufs=1))
            stat = ctx.enter_context(tc.tile_pool(name="stat", bufs=1))
            small = ctx.enter_context(tc.tile_pool(name="small", bufs=4))
            sqp = ctx.enter_context(tc.tile_pool(name="sqp", bufs=3))
            ynp = ctx.enter_context(tc.tile_pool(name="ynp", bufs=3))
            g2p = ctx.enter_context(tc.tile_pool(name="g2p", bufs=2))
            outp = ctx.enter_context(tc.tile_pool(name="outp", bufs=3))

            # ---- input DMAs (x of batch 0 first: it gates the start) --
            x_sb = []
            for b in range(BL):
                xt = big.tile([P, NC_TOK, H], BF16, name=f"x{b}")
                nc.sync.dma_start(xt, x_v[b])
                x_sb.append(xt)
            g1_sb = const.tile([P, 1], F32)
            nc.sync.dma_start(g1_sb, g1_d[:])
            cw1_sb = const.tile([H, CM], BF16)
            nc.sync.dma_start(cw1_sb, cw1_d[:])
            cb1_sb = const.tile([P, NCI], F32)
            nc.sync.dma_start(cb1_sb, cb1_d[:])
            cw2_sb = const.tile([P, NCI, H], BF16)
            nc.sync.dma_start(cw2_sb, cw2_v)
            onesc_sb = const.tile([P, 1], BF16)
            nc.sync.dma_start(onesc_sb, onesc_d[:])
            ident = const.tile([P, P], BF16)
            make_identity(nc, ident)
            if nontrivial_bias1:
                bias1_sb = const.tile([P, TM], F32)
                nc.sync.dma_start(bias1_sb, bias1_d[:])
            if nontrivial_cb2:
                cb2_sb = const.tile([P, 1], F32)
                nc.sync.dma_start(cb2_sb, cb2_d[:])
            w1c_sb = big.tile([P, NC_TOK, TM], BF16, name="w1c")
            nc.sync.dma_start(w1c_sb, w1c_v)
            w2_sb = big.tile([P, KTM, NJ, JW], BF16, name="w2")
            nc.sync.dma_start(w2_sb, w2_v)
            g1_t = small.tile([P, 1], F32)
            nc.vector.tensor_copy(g1_t, g1_sb)

            y2r = [big.tile([P, N], BF16, name=f"y2r{b}")
                   for b in range(BL)]
            bcast = [big.tile([P, N], BF16, name=f"bc{b}")
                     for b in range(BL)]

            # ---------------------------------------------------------
            def rsqrt_chain(dst, vsrc, tmp_a, tmp_b):
                """dst = 1/sqrt(vsrc) (vsrc f32 SBUF [P, n]).

                Fast-inverse-sqrt seed (int ops on DVE: pool rejects the
                TensorScalarPtr opcode, and the DVE's fp32-ALU int path
                is accurate to ~1e-6 here) + 2 Newton steps with the
                multiplies on GPSIMD. tmp_a/tmp_b f32 same shape; dst
                may be bf16.
                """
                iv = vsrc[:].bitcast(I32)
                nc.vector.tensor_scalar(
                    out=tmp_a[:].bitcast(I32), in0=iv, scalar1=-1, scalar2=1,
                    op0=ALU.bitwise_xor, op1=ALU.arith_shift_right)
                nc.vector.tensor_scalar(
                    out=tmp_b[:].bitcast(I32), in0=tmp_a[:].bitcast(I32),
                    scalar1=MAGIC1, scalar2=None, op0=ALU.add)
                for it in range(2):
                    nc.gpsimd.tensor_tensor(tmp_a, tmp_b, tmp_b, ALU.mult)
                    nc.gpsimd.tensor_tensor(tmp_a, tmp_a, vsrc, ALU.mult)
                    nc.vector.tensor_scalar(
                        out=tmp_a, in0=tmp_a, scalar1=-0.5, scalar2=1.5,
                        op0=ALU.mult, op1=ALU.add)
                    nc.gpsimd.tensor_tensor(
                        dst if it == 1 else tmp_b, tmp_b, tmp_a, ALU.mult)

            # ---- LN1 stats (ACT square + DVE reduces + GPSIMD rsqrt) --
            mu1 = []
            rstd1 = []
            stats_sc = []
            for b in range(BL):
                mu = stat.tile([P, NC_TOK], F32, name=f"mu{b}")
                vs = stat.tile([P, NC_TOK], F32, name=f"vs{b}")
                ta = stat.tile([P, NC_TOK], F32, name=f"ta{b}")
                tb = stat.tile([P, NC_TOK], F32, name=f"tb{b}")
                rs = stat.tile([P, NC_TOK], F32, name=f"rs{b}")
                mu1.append(mu)
                rstd1.append(rs)
                stats_sc.append((vs, ta, tb))

            def emit_stats(b):
                vs, ta, tb = stats_sc[b]
                # y2r[b] is unused until phase 3a -> borrow it as the
                # square scratch for the LN1 sum-of-squares pass
                sq = y2r[b][:].rearrange("p (c h) -> p c h", h=H)
                nc.scalar.activation(sq, x_sb[b], AF.Square)
                nc.vector.tensor_reduce(
                    out=ta, in_=x_sb[b], axis=AX.X, op=ALU.add)
                nc.vector.tensor_reduce(
                    out=tb, in_=sq, axis=AX.X, op=ALU.add)
                # mu = sums/H ; var+eps = sumsq/H + EPS - mu^2
                nc.vector.tensor_scalar_mul(mu1[b], ta, 1.0 / H)
                nc.vector.tensor_scalar(
                    out=tb, in0=tb, scalar1=1.0 / H, scalar2=EPS,
                    op0=ALU.mult, op1=ALU.add)
                nc.vector.tensor_tensor(ta, mu1[b], mu1[b], ALU.mult)
                nc.vector.tensor_tensor(vs, tb, ta, ALU.subtract)
                rsqrt_chain(rstd1[b], vs, ta, tb)

            with (
                tc.tile_pool(name="ps2", bufs=2, space="PSUM") as ps2,
                tc.tile_pool(name="vcp", bufs=1, space="PSUM") as vcp,
            ):
                vc = vcp.tile([P, BL * NC_TOK], F32, name="vc")
                h1c = [[None] * KTM for _ in range(BL)]
                sq_tiles = {}

                def emit_mm1(b, ps1):
                    psum1 = ps1.tile([P, TM], F32, tag="ps1")
                    for c in range(NC_TOK):
                        xn = small.tile([P, P], BF16, tag="xn")
                        nc.vector.tensor_scalar(
                            out=xn,
                            in0=x_sb[b][:, c, :],
                            scalar1=mu1[b][:, c:c + 1],
                            scalar2=rstd1[b][:, c:c + 1],
                            op0=ALU.subtract,
                            op1=ALU.mult,
                        )
                        nc.tensor.matmul(
                            psum1, xn, w1c_sb[:, c, :],
                            start=(c == 0), stop=(c == NC_TOK - 1))
                    return psum1

                def emit_ph2(b, psum1, pst):
                    h1 = small.tile([P, TM], BF16, tag="h1")
                    if nontrivial_bias1:
                        h1f = small.tile([P, TM], F32, tag="h1f")
                        nc.vector.tensor_scalar_mul(h1f, psum1, g1_t)
                        nc.vector.tensor_add(h1f, h1f, bias1_sb)
                        nc.scalar.activation(h1, h1f, AF.Gelu)
                    else:
                        nc.scalar.activation(h1, psum1, AF.Gelu, scale=g1_t)
                    for k in range(KTM):
                        hk = h1[:, k * P:(k + 1) * P]
                        ps_t = pst.tile([P, P], BF16, tag="pst")
                        nc.tensor.transpose(ps_t, hk, ident)
                        hm = pst.tile([P, 1], F32, tag="hm")
                        nc.tensor.matmul(hm, hk, onesc_sb,
                                         start=True, stop=True)
                        nhm = small.tile([P, 1], F32, tag="nhm")
                        nc.scalar.activation(nhm, hm, AF.Copy,
                                             scale=float(-1.0 / H))
                        hc = small.tile([P, P], BF16, tag="h1c",
                                        name=f"hc{b}_{k}")
                        nc.scalar.activation(hc, ps_t, AF.Identity,
                                             bias=nhm)
                        h1c[b][k] = hc

                def emit_3a_mm(b, j, y2r_engine):
                    """token matmul 2 for (b, j) + psum drain + square."""
                    p2 = ps2.tile([P, JW], F32, tag="ps2")
                    for k in range(KTM):
                        nc.tensor.matmul(
                            p2, h1c[b][k], w2_sb[:, k, j, :],
                            start=(k == 0), stop=(k == KTM - 1))
                    yj = y2r[b][:, j * JW:(j + 1) * JW]
                    if y2r_engine == "act":
                        nc.scalar.activation(yj, p2, AF.Copy)
                    else:
                        nc.vector.tensor_copy(yj, p2)
                    sq = sqp.tile([P, JW], BF16, tag="sq")
                    nc.gpsimd.tensor_tensor(sq, yj, yj, ALU.mult)
                    sq_tiles[(b, j)] = sq

                def emit_3a_vc(b, j):
                    """compact per-token variance columns for (b, j)."""
                    sq = sq_tiles.pop((b, j))
                    for c in range(TPJ):
                        col = b * NC_TOK + j * TPJ + c
                        nc.tensor.matmul(
                            vc[:, col:col + 1],
                            sq[:, c * P:(c + 1) * P], onesc_sb,
                            start=True, stop=True)

                def emit_chain2(b):
                    vs = stat.tile([P, NC_TOK], F32, name=f"vs2_{b}")
                    ta = stat.tile([P, NC_TOK], F32, name=f"ta2_{b}")
                    tb = stat.tile([P, NC_TOK], F32, name=f"tb2_{b}")
                    # padded to [P, P]: the XBAR transpose DMA needs a
                    # multiple-of-128 free dim
                    rc = stat.tile([P, P], BF16, name=f"rc2_{b}")
                    nc.gpsimd.memset(rc[:, NC_TOK:], 0.0)
                    nc.vector.tensor_scalar(
                        out=vs, in0=vc[:, b * NC_TOK:(b + 1) * NC_TOK],
                        scalar1=1.0 / H, scalar2=EPS,
                        op0=ALU.mult, op1=ALU.add)
                    rsqrt_chain(rc[:, 0:NC_TOK], vs, ta, tb)
                    # rc[t, c] --plain--> rs1 --transposed--> rr[c, t]
                    #   --plain--> rs2 --stride-0 broadcast--> bcast[p, c*t]
                    nc.sync.dma_start(rs1_d[b], rc)
                    rr = small.tile([P, P], BF16, tag="rr")
                    nc.sync.dma_start(rr, rs1_d[b], transpose=True)
                    nc.sync.dma_start(rs2_d[b], rr[0:NC_TOK, :])
                    nc.sync.dma_start(
                        bcast[b],
                        rs2_d[b].rearrange(
                            "c t -> (c t)").partition_broadcast(P))

                def emit_3b(b, j, psr, drain_engine):
                    yn = ynp.tile([P, JW], BF16, tag="yn")
                    nc.vector.tensor_tensor(
                        yn, y2r[b][:, j * JW:(j + 1) * JW],
                        bcast[b][:, j * JW:(j + 1) * JW], ALU.mult)
                    raw = psr.tile([P, NCI * JW], F32, tag="psr")
                    for ci in range(NCI):
                        nc.tensor.matmul(
                            raw[:, ci * JW:(ci + 1) * JW],
                            cw1_sb[:, ci * P:(ci + 1) * P],
                            yn, start=True, stop=True)
                    g2 = g2p.tile([P, NCI * JW], BF16, tag="g2")
                    if nontrivial_bias1:
                        for ci in range(NCI):
                            nc.scalar.activation(
                                g2[:, ci * JW:(ci + 1) * JW],
                                raw[:, ci * JW:(ci + 1) * JW],
                                AF.Gelu, bias=cb1_sb[:, ci:ci + 1])
                    else:
                        nc.scalar.activation(g2, raw, AF.Gelu)
                    # accumulate the output into bank 0 of `raw` (it has
                    # been fully consumed by the gelu above) to stay
                    # within the 8-bank PSUM budget.
                    po = raw[:, 0:JW]
                    for ci in range(NCI):
                        nc.tensor.matmul(
                            po, cw2_sb[:, ci, :],
                            g2[:, ci * JW:(ci + 1) * JW],
                            start=(ci == 0), stop=(ci == NCI - 1))
                    osb = outp.tile([P, JW], F32, tag="osb")
                    if drain_engine == "act":
                        if nontrivial_cb2:
                            nc.scalar.activation(osb, po, AF.Identity,
                                                 bias=cb2_sb)
                        else:
                            nc.scalar.activation(osb, po, AF.Copy)
                    else:
                        if nontrivial_cb2:
                            nc.vector.tensor_scalar(
                                out=osb, in0=po, scalar1=cb2_sb,
                                scalar2=None, op0=ALU.add)
                        else:
                            nc.vector.tensor_copy(osb, po)
                    nc.sync.dma_start(out_v[b][:, j * JW:(j + 1) * JW], osb)

                # ---- emission schedule --------------------------------
                with (
                    tc.tile_pool(name="ps1", bufs=1, space="PSUM") as ps1,
                    tc.tile_pool(name="pst", bufs=2, space="PSUM") as pst,
                ):
                    emit_stats(0)
                    p1_0 = emit_mm1(0, ps1)
                    emit_ph2(0, p1_0, pst)
                    emit_stats(1)
                    # batch-0 token-mm2 sweep (vc staggered one behind)
                    for j in range(NJ):
                        emit_3a_mm(0, j, "act")
                        if j > 0:
                            emit_3a_vc(0, j - 1)
                    emit_3a_vc(0, NJ - 1)
                    p1_1 = emit_mm1(1, ps1)
                    emit_ph2(1, p1_1, pst)
                    emit_chain2(0)

                with tc.tile_pool(name="psrA", bufs=1, space="PSUM") as psrA:
                    # batch-0 channel MLP interleaved with batch-1 mm2
                    for j in range(NJ):
                        emit_3b(0, j, psrA, "act" if j % 2 else "dve")
                        emit_3a_mm(1, j, "dve")
                        if j > 0:
                            emit_3a_vc(1, j - 1)
                    emit_3a_vc(1, NJ - 1)
                    emit_chain2(1)

            # tail: batch-1 channel MLP, double-buffered psum (all other
            # psum pools are closed by now so two 4-bank tiles fit)
            with (
                tc.tile_pool(name="psrB", bufs=1, space="PSUM") as psrB,
                tc.tile_pool(name="psrC", bufs=1, space="PSUM") as psrC,
            ):
                for j in range(NJ):
                    emit_3b(1, j, psrB if j % 2 else psrC,
                            "act" if j % 2 else "dve")

    nc.compile()
    return nc


def _host_prep(inputs):
    import ml_dtypes

    x = np.asarray(inputs["x"], dtype=np.float32)
    ln1_g = np.asarray(inputs["ln1_g"], np.float32)
    ln1_b = np.asarray(inputs["ln1_b"], np.float32)
    ln2_g = np.asarray(inputs["ln2_g"], np.float32)
    ln2_b = np.asarray(inputs["ln2_b"], np.float32)
    tok_w1 = np.asarray(inputs["tok_w1"], np.float32)
    tok_b1 = np.asarray(inputs["tok_b1"], np.float32)
    tok_w2 = np.asarray(inputs["tok_w2"], np.float32)
    ch_w1 = np.asarray(inputs["ch_w1"], np.float32)
    ch_b1 = np.asarray(inputs["ch_b1"], np.float32)
    ch_w2 = np.asarray(inputs["ch_w2"], np.float32)
    ch_b2 = np.asarray(inputs["ch_b2"], np.float32)

    BF = ml_dtypes.bfloat16
    w1c = np.cumsum(tok_w1, axis=0, dtype=np.float64).astype(np.float32)
    colsum1 = w1c.sum(axis=0, dtype=np.float64).astype(np.float32)
    bias1 = ln1_b[:, None] * colsum1[None, :] + tok_b1[None, :]
    cw1 = (ln2_g[:, None] * ch_w1).astype(np.float32)
    cb1 = (ch_b1 + ch_w1.T @ ln2_b).astype(np.float32)

    nontrivial_bias1 = bool(np.any(bias1 != 0.0) or np.any(cb1 != 0.0))
    nontrivial_cb2 = bool(np.any(ch_b2 != 0.0))

    shared = {
        "w1c": w1c.astype(BF),
        "w2": np.ascontiguousarray(tok_w2).astype(BF),
        "g1": ln1_g.reshape(P, 1).copy(),
        "bias1": np.ascontiguousarray(bias1, np.float32),
        "cw1": cw1.astype(BF),
        "cb1": np.ascontiguousarray(cb1.reshape(NCI, P).T.copy()),
        "cw2": np.ascontiguousarray(ch_w2).astype(BF),
        "cb2": ch_b2.reshape(P, 1).astype(np.float32).copy(),
        "onesc": np.ones((P, 1), BF),
    }
    return x.astype(BF), shared, nontrivial_bias1, nontrivial_cb2


def kernel(**inputs) -> np.ndarray:
    from concourse.bass_utils import run_bass_kernel_spmd

    x, shared, nb1, nb2 = _host_prep(inputs)

    key = (nb1, nb2)
    if key not in _cached:
        _cached[key] = _build(nb1, nb2)
    nc = _cached[key]

    in_maps = []
    for c in range(NCORES):
        m = dict(shared)
        m["x"] = np.ascontiguousarray(x[c * BL:(c + 1) * BL])
        in_maps.append(m)

    res = run_bass_kernel_spmd(nc, in_maps, core_ids=list(range(NCORES)))
    out = np.concatenate(
        [r["out"].transpose(0, 2, 1) for r in res.results], axis=0)
    return np.ascontiguousarray(out, dtype=np.float32)


if __name__ == "__main__":
    rng = np.random.default_rng(0)
    ins = {
        "x": rng.standard_normal((B, N, H)).astype(np.float32),
        "ln1_g": np.ones(H, np.float32),
        "ln1_b": np.zeros(H, np.float32),
        "ln2_g": np.ones(H, np.float32),
        "ln2_b": np.zeros(H, np.float32),
        "tok_w1": (rng.standard_normal((N, TM)) * 0.02).astype(np.float32),
        "tok_b1": np.zeros(TM, np.float32),
        "tok_w2": (rng.standard_normal((TM, N)) * 0.02).astype(np.float32),
        "tok_b2": np.zeros(N, np.float32),
        "ch_w1": (rng.standard_normal((H, CM)) * 0.02).astype(np.float32),
        "ch_b1": np.zeros(CM, np.float32),
        "ch_w2": (rng.standard_normal((CM, H)) * 0.02).astype(np.float32),
        "ch_b2": np.zeros(H, np.float32),
    }
    out = kernel(**ins)
    print("out", out.shape, out.dtype)


# revision 19
# speedup vs baseline: 1.4215x; 1.0954x over previous
"""Trainium2 Bass kernel for nn_AutoregressiveMixerBlock.

Reference computation (per batch b):
  y  = LN_H(x)                                    # layer norm over H
  t  = revcumsum_N(y)                             # t[j] = sum_{i>=j} y[i]
  h  = gelu(t^T @ tok_w1 + tok_b1)                # [H, TM]
  y2 = (h @ tok_w2 + tok_b2)^T                    # [N, H]
  y3 = LN_H(y2)
  out = gelu(y3 @ ch_w1 + ch_b1) @ ch_w2 + ch_b2  # [N, H]

Algebraic folds (exact in real arithmetic, applied on host):
  * revcumsum+matmul:  sum_j t[j,h] w1[j,m] = sum_i y[i,h] W1c[i,m]
    with W1c = cumsum(tok_w1, axis=0) -> no on-device cumsum at all.
  * LN1 gain/bias move past the token matmul.
  * tok_b2 and the LN2 mean both vanish by centering h^T by its
    per-row (over H) mean before the second token matmul.
  * LN2 gain/bias fold into ch_w1 / ch_b1.

Device strategy (per core, 2 batches):
  * all matmul operands bf16 (PE 1 cyc/row), f32 PSUM accumulate.
  * LN1 stats WITHOUT vector-engine reductions: a DMA-transposed copy
    of x ([H, N] layout) is squared on the scalar engine, and both
    sum_h and sum_h^2 per token come from tiny ones-column matmuls
    (contraction over partitions = H) into compact psum columns.
  * rsqrt everywhere via the fast-inverse-sqrt int hack (int ops on
    DVE, Newton multiplies on GPSIMD) -> the scalar engine only ever
    runs Gelu/Square/Copy/Identity = ONE activation table set.
  * LN2 variance reduced the same way (ones-column matmuls on the
    squared token-mix output); the per-token rstd is broadcast to a
    [128, N] bf16 tile via a DRAM roundtrip (store / transposed load /
    store / stride-0 broadcast load).
  * channel MLP output accumulates into bank 0 of the gelu-input psum
    tile (already consumed) and the two batches' channel-MLP sweeps
    run back-to-back with two alternating 4-bank psum tiles.
"""

import numpy as np

B, N, H = 16, 8192, 128
TM, CM = 256, 512
EPS = 1e-5
NCORES = 8
BL = B // NCORES          # batches per core
P = 128                   # partitions
NC_TOK = N // P           # 64 token chunks of 128
NJ = 16                   # j chunks per batch
JW = N // NJ              # 512 tokens per j chunk
TPJ = JW // P             # 4 token chunks per j chunk
KTM = TM // P             # 2 k-chunks for the second token matmul
NCI = CM // P             # 4 chunks of the channel hidden dim
MAGIC1 = 0x5F3759DF + 1   # fast-rsqrt magic (for the xor/asr variant)

_cached = {}


def _build(nontrivial_bias1, nontrivial_cb2):
    import contextlib

    import concourse.mybir as mybir
    import concourse.tile as tile
    from concourse import bacc
    from concourse.masks import make_identity

    F32 = mybir.dt.float32
    BF16 = mybir.dt.bfloat16
    I32 = mybir.dt.int32
    AF = mybir.ActivationFunctionType
    ALU = mybir.AluOpType

    nc = bacc.Bacc()

    # ---- DRAM tensors -------------------------------------------------
    x_d = nc.dram_tensor("x", [BL, N, H], BF16, kind="ExternalInput")
    w1c_d = nc.dram_tensor("w1c", [N, TM], BF16, kind="ExternalInput")
    w2_d = nc.dram_tensor("w2", [TM, N], BF16, kind="ExternalInput")
    g1_d = nc.dram_tensor("g1", [P, 1], F32, kind="ExternalInput")
    bias1_d = nc.dram_tensor("bias1", [P, TM], F32, kind="ExternalInput")
    cw1_d = nc.dram_tensor("cw1", [H, CM], BF16, kind="ExternalInput")
    cb1_d = nc.dram_tensor("cb1", [P, NCI], F32, kind="ExternalInput")
    cw2_d = nc.dram_tensor("cw2", [CM, H], BF16, kind="ExternalInput")
    cb2_d = nc.dram_tensor("cb2", [P, 1], F32, kind="ExternalInput")
    onesc_d = nc.dram_tensor("onesc", [P, 1], BF16, kind="ExternalInput")
    # rstd scratch: [t, c] then [c, t] layouts (DMA-transposed between)
    rs1_d = nc.dram_tensor("rs1", [BL, P, P], BF16, kind="ExternalOutput")
    rs2_d = nc.dram_tensor("rs2", [BL, NC_TOK, P], BF16,
                           kind="ExternalOutput")
    out_d = nc.dram_tensor("out", [BL, H, N], F32, kind="ExternalOutput")

    # DRAM views
    x_v = [x_d[b].rearrange("(c p) h -> p c h", p=P) for b in range(BL)]
    w1c_v = w1c_d[:].rearrange("(c p) m -> p c m", p=P)
    w2_v = w2_d[:].rearrange("(k p) (j n) -> p k j n", p=P, n=JW)
    cw2_v = cw2_d[:].rearrange("(ci p) h -> p ci h", p=P)
    out_v = [out_d[b] for b in range(BL)]

    with tile.TileContext(nc) as tc:
        with contextlib.ExitStack() as ctx:
            const = ctx.enter_context(tc.tile_pool(name="const", bufs=1))
            big = ctx.enter_context(tc.tile_pool(name="big", bufs=1))
            stat = ctx.enter_context(tc.tile_pool(name="stat", bufs=1))
            small = ctx.enter_context(tc.tile_pool(name="small", bufs=4))
            w2s = ctx.enter_context(tc.tile_pool(name="w2s", bufs=6))
            sqp = ctx.enter_context(tc.tile_pool(name="sqp", bufs=3))
            ynp = ctx.enter_context(tc.tile_pool(name="ynp", bufs=3))
            g2p = ctx.enter_context(tc.tile_pool(name="g2p", bufs=2))
            outp = ctx.enter_context(tc.tile_pool(name="outp", bufs=3))

            # big retained tiles; bcast[b] doubles as the transposed-x
            # scratch and y2r[b] as the squared-x scratch during LN1
            # (both are otherwise unused until phase 3)
            y2r = [big.tile([P, N], BF16, name=f"y2r{b}")
                   for b in range(BL)]
            bcast = [big.tile([P, N], BF16, name=f"bc{b}")
                     for b in range(BL)]
            xT_sb = bcast

            # ---- input DMAs (batch-0 x first: it gates the start) -----
            x_sb = []
            for b in range(BL):
                xt = big.tile([P, NC_TOK, H], BF16, name=f"x{b}")
                nc.sync.dma_start(xt, x_v[b])
                x_sb.append(xt)
                nc.sync.dma_start(xT_sb[b], x_d[b], transpose=True)
            g1_sb = const.tile([P, 1], F32)
            nc.sync.dma_start(g1_sb, g1_d[:])
            cw1_sb = const.tile([H, CM], BF16)
            nc.sync.dma_start(cw1_sb, cw1_d[:])
            cb1_sb = const.tile([P, NCI], F32)
            nc.sync.dma_start(cb1_sb, cb1_d[:])
            cw2_sb = const.tile([P, NCI, H], BF16)
            nc.sync.dma_start(cw2_sb, cw2_v)
            onesc_sb = const.tile([P, 1], BF16)
            nc.sync.dma_start(onesc_sb, onesc_d[:])
            ident = const.tile([P, P], BF16)
            make_identity(nc, ident)
            if nontrivial_bias1:
                bias1_sb = const.tile([P, TM], F32)
                nc.sync.dma_start(bias1_sb, bias1_d[:])
            if nontrivial_cb2:
                cb2_sb = const.tile([P, 1], F32)
                nc.sync.dma_start(cb2_sb, cb2_d[:])
            w1c_sb = big.tile([P, NC_TOK, TM], BF16, name="w1c")
            nc.sync.dma_start(w1c_sb, w1c_v)
            g1_t = small.tile([P, 1], F32)
            nc.vector.tensor_copy(g1_t, g1_sb)

            # ---------------------------------------------------------
            def rsqrt_chain(dst, vsrc, tmp_a, tmp_b):
                """dst = 1/sqrt(vsrc) (vsrc f32 SBUF [P, n]).

                Fast-inverse-sqrt seed (int ops on DVE; its fp32-ALU int
                path is accurate to ~1e-6 here, and pool rejects the
                TensorScalarPtr opcode) + 2 Newton steps with the
                multiplies on GPSIMD. tmp_a/tmp_b f32 same shape; dst
                may be bf16.
                """
                iv = vsrc[:].bitcast(I32)
                nc.vector.tensor_scalar(
                    out=tmp_a[:].bitcast(I32), in0=iv, scalar1=-1, scalar2=1,
                    op0=ALU.bitwise_xor, op1=ALU.arith_shift_right)
                nc.vector.tensor_scalar(
                    out=tmp_b[:].bitcast(I32), in0=tmp_a[:].bitcast(I32),
                    scalar1=MAGIC1, scalar2=None, op0=ALU.add)
                for it in range(2):
                    nc.gpsimd.tensor_tensor(tmp_a, tmp_b, tmp_b, ALU.mult)
                    nc.gpsimd.tensor_tensor(tmp_a, tmp_a, vsrc, ALU.mult)
                    nc.vector.tensor_scalar(
                        out=tmp_a, in0=tmp_a, scalar1=-0.5, scalar2=1.5,
                        op0=ALU.mult, op1=ALU.add)
                    nc.gpsimd.tensor_tensor(
                        dst if it == 1 else tmp_b, tmp_b, tmp_a, ALU.mult)

            mu1 = []
            rstd1 = []
            nmr1 = []
            for b in range(BL):
                mu1.append(stat.tile([P, NC_TOK], F32, name=f"mu{b}"))
                rstd1.append(stat.tile([P, NC_TOK], F32, name=f"rs{b}"))
                nmr1.append(stat.tile([P, NC_TOK], F32, name=f"nm{b}"))

            with (
                tc.tile_pool(name="pstat", bufs=1, space="PSUM") as pstat,
                tc.tile_pool(name="ps1", bufs=1, space="PSUM") as ps1,
                tc.tile_pool(name="pst", bufs=1, space="PSUM") as pst,
                tc.tile_pool(name="ps2", bufs=2, space="PSUM") as ps2,
                tc.tile_pool(name="vcp", bufs=1, space="PSUM") as vcp,
            ):
                vc = vcp.tile([P, BL * NC_TOK], F32, name="vc")
                h1c = [[None] * KTM for _ in range(BL)]
                sq_tiles = {}

                def emit_stats(b):
                    """LN1 per-token mean/rstd without DVE reductions."""
                    sqT = y2r[b]
                    nc.scalar.activation(sqT, xT_sb[b], AF.Square)
                    ps = pstat.tile([P, P], F32, tag="pstat")
                    for c in range(NC_TOK):
                        nc.tensor.matmul(
                            ps[:, c:c + 1],
                            xT_sb[b][:, c * P:(c + 1) * P], onesc_sb,
                            start=True, stop=True)
                    for c in range(NC_TOK):
                        nc.tensor.matmul(
                            ps[:, NC_TOK + c:NC_TOK + c + 1],
                            sqT[:, c * P:(c + 1) * P], onesc_sb,
                            start=True, stop=True)
                    vs = stat.tile([P, NC_TOK], F32, name=f"vs1_{b}")
                    ta = stat.tile([P, NC_TOK], F32, name=f"ta1_{b}")
                    tb = stat.tile([P, NC_TOK], F32, name=f"tb1_{b}")
                    # mu = sums/H ; var+eps = sumsq/H + EPS - mu^2
                    nc.vector.tensor_scalar_mul(mu1[b], ps[:, 0:NC_TOK],
                                                1.0 / H)
                    nc.vector.tensor_scalar(
                        out=ta, in0=ps[:, NC_TOK:], scalar1=1.0 / H,
                        scalar2=EPS, op0=ALU.mult, op1=ALU.add)
                    nc.vector.tensor_tensor(tb, mu1[b], mu1[b], ALU.mult)
                    nc.vector.tensor_tensor(vs, ta, tb, ALU.subtract)
                    rsqrt_chain(rstd1[b], vs, ta, tb)
                    # -mu*rstd, the per-chunk bias for the ACT xn path
                    nc.vector.tensor_tensor(ta, mu1[b], rstd1[b], ALU.mult)
                    nc.vector.tensor_scalar_mul(nmr1[b], ta, -1.0)

                def emit_mm1(b):
                    psum1 = ps1.tile([P, TM], F32, tag="ps1")
                    for c in range(NC_TOK):
                        xn = small.tile([P, P], BF16, tag="xn")
                        if c % 2 == 0:
                            nc.vector.tensor_scalar(
                                out=xn,
                                in0=x_sb[b][:, c, :],
                                scalar1=mu1[b][:, c:c + 1],
                                scalar2=rstd1[b][:, c:c + 1],
                                op0=ALU.subtract,
                                op1=ALU.mult,
                            )
                        else:
                            nc.scalar.activation(
                                xn, x_sb[b][:, c, :], AF.Identity,
                                bias=nmr1[b][:, c:c + 1],
                                scale=rstd1[b][:, c:c + 1])
                        nc.tensor.matmul(
                            psum1, xn, w1c_sb[:, c, :],
                            start=(c == 0), stop=(c == NC_TOK - 1))
                    return psum1

                def emit_ph2(b, psum1):
                    h1 = small.tile([P, TM], BF16, tag="h1")
                    if nontrivial_bias1:
                        h1f = small.tile([P, TM], F32, tag="h1f")
                        nc.vector.tensor_scalar_mul(h1f, psum1, g1_t)
                        nc.vector.tensor_add(h1f, h1f, bias1_sb)
                        nc.scalar.activation(h1, h1f, AF.Gelu)
                    else:
                        nc.scalar.activation(h1, psum1, AF.Gelu, scale=g1_t)
                    for k in range(KTM):
                        hk = h1[:, k * P:(k + 1) * P]
                        ps_t = pst.tile([P, P], BF16, tag="pst")
                        nc.tensor.transpose(ps_t, hk, ident)
                        hm = pst.tile([P, 1], F32, tag="hm")
                        nc.tensor.matmul(hm, hk, onesc_sb,
                                         start=True, stop=True)
                        nhm = small.tile([P, 1], F32, tag="nhm")
                        nc.scalar.activation(nhm, hm, AF.Copy,
                                             scale=float(-1.0 / H))
                        hc = small.tile([P, P], BF16, tag="h1c",
                                        name=f"hc{b}_{k}")
                        nc.scalar.activation(hc, ps_t, AF.Identity,
                                             bias=nhm)
                        h1c[b][k] = hc

                def emit_3a_mm(b, j, y2r_engine, sq_engine):
                    """token matmul 2 for (b, j) + psum drain + square."""
                    w2t = []
                    for k in range(KTM):
                        wt = w2s.tile([P, JW], BF16, tag="w2")
                        nc.sync.dma_start(wt, w2_v[:, k, j, :])
                        w2t.append(wt)
                    p2 = ps2.tile([P, JW], F32, tag="ps2")
                    for k in range(KTM):
                        nc.tensor.matmul(
                            p2, h1c[b][k], w2t[k],
                            start=(k == 0), stop=(k == KTM - 1))
                    yj = y2r[b][:, j * JW:(j + 1) * JW]
                    if y2r_engine == "act":
                        nc.scalar.activation(yj, p2, AF.Copy)
                    else:
                        nc.vector.tensor_copy(yj, p2)
                    sq = sqp.tile([P, JW], BF16, tag="sq")
                    if sq_engine == "gpsimd":
                        nc.gpsimd.tensor_tensor(sq, yj, yj, ALU.mult)
                    else:
                        nc.vector.tensor_tensor(sq, yj, yj, ALU.mult)
                    sq_tiles[(b, j)] = sq

                def emit_3a_vc(b, j):
                    """compact per-token variance columns for (b, j)."""
                    sq = sq_tiles.pop((b, j))
                    for c in range(TPJ):
                        col = b * NC_TOK + j * TPJ + c
                        nc.tensor.matmul(
                            vc[:, col:col + 1],
                            sq[:, c * P:(c + 1) * P], onesc_sb,
                            start=True, stop=True)

                def emit_chain2(b):
                    vs = stat.tile([P, NC_TOK], F32, name=f"vs2_{b}")
                    ta = stat.tile([P, NC_TOK], F32, name=f"ta2_{b}")
                    tb = stat.tile([P, NC_TOK], F32, name=f"tb2_{b}")
                    # padded to [P, P]: the XBAR transpose DMA needs a
                    # multiple-of-128 free dim
                    rc = stat.tile([P, P], BF16, name=f"rc2_{b}")
                    nc.gpsimd.memset(rc[:, NC_TOK:], 0.0)
                    nc.vector.tensor_scalar(
                        out=vs, in0=vc[:, b * NC_TOK:(b + 1) * NC_TOK],
                        scalar1=1.0 / H, scalar2=EPS,
                        op0=ALU.mult, op1=ALU.add)
                    rsqrt_chain(rc[:, 0:NC_TOK], vs, ta, tb)
                    # rc[t, c] --plain--> rs1 --transposed--> rr[c, t]
                    #   --plain--> rs2 --stride-0 broadcast--> bcast[p, c*t]
                    nc.sync.dma_start(rs1_d[b], rc)
                    rr = small.tile([P, P], BF16, tag="rr")
                    nc.sync.dma_start(rr, rs1_d[b], transpose=True)
                    nc.sync.dma_start(rs2_d[b], rr[0:NC_TOK, :])
                    nc.sync.dma_start(
                        bcast[b],
                        rs2_d[b].rearrange(
                            "c t -> (c t)").partition_broadcast(P))

                def emit_3b(b, j, psr):
                    yn = ynp.tile([P, JW], BF16, tag="yn")
                    nc.vector.tensor_tensor(
                        yn, y2r[b][:, j * JW:(j + 1) * JW],
                        bcast[b][:, j * JW:(j + 1) * JW], ALU.mult)
                    raw = psr.tile([P, NCI * JW], F32, tag="psr")
                    for ci in range(NCI):
                        nc.tensor.matmul(
                            raw[:, ci * JW:(ci + 1) * JW],
                            cw1_sb[:, ci * P:(ci + 1) * P],
                            yn, start=True, stop=True)
                    g2 = g2p.tile([P, NCI * JW], BF16, tag="g2")
                    if nontrivial_bias1:
                        for ci in range(NCI):
                            nc.scalar.activation(
                                g2[:, ci * JW:(ci + 1) * JW],
                                raw[:, ci * JW:(ci + 1) * JW],
                                AF.Gelu, bias=cb1_sb[:, ci:ci + 1])
                    else:
                        nc.scalar.activation(g2, raw, AF.Gelu)
                    # accumulate the output into bank 0 of `raw` (fully
                    # consumed by the gelu above) to stay in 8 banks
                    po = raw[:, 0:JW]
                    for ci in range(NCI):
                        nc.tensor.matmul(
                            po, cw2_sb[:, ci, :],
                            g2[:, ci * JW:(ci + 1) * JW],
                            start=(ci == 0), stop=(ci == NCI - 1))
                    osb = outp.tile([P, JW], F32, tag="osb")
                    if nontrivial_cb2:
                        nc.vector.tensor_scalar(
                            out=osb, in0=po, scalar1=cb2_sb,
                            scalar2=None, op0=ALU.add)
                    else:
                        nc.vector.tensor_copy(osb, po)
                    nc.sync.dma_start(out_v[b][:, j * JW:(j + 1) * JW], osb)

                # ---- emission schedule --------------------------------
                emit_stats(0)
                p1_0 = emit_mm1(0)
                emit_ph2(0, p1_0)
                emit_stats(1)
                for j in range(NJ):
                    emit_3a_mm(0, j, "act", "gpsimd")
                    if j > 0:
                        emit_3a_vc(0, j - 1)
                emit_3a_vc(0, NJ - 1)
                emit_chain2(0)
                p1_1 = emit_mm1(1)
                emit_ph2(1, p1_1)
                for j in range(NJ):
                    emit_3a_mm(1, j, "act", "dve")
                    if j > 0:
                        emit_3a_vc(1, j - 1)
                emit_3a_vc(1, NJ - 1)
                emit_chain2(1)

            # channel MLP for both batches, double-buffered 4-bank psum
            with (
                tc.tile_pool(name="psrA", bufs=1, space="PSUM") as psrA,
                tc.tile_pool(name="psrB", bufs=1, space="PSUM") as psrB,
            ):
                for b in range(BL):
                    for j in range(NJ):
                        emit_3b(b, j, psrA if (b * NJ + j) % 2 else psrB)

    nc.compile()
    return nc


def _host_prep(inputs):
    import ml_dtypes

    x = np.asarray(inputs["x"], dtype=np.float32)
    ln1_g = np.asarray(inputs["ln1_g"], np.float32)
    ln1_b = np.asarray(inputs["ln1_b"], np.float32)
    ln2_g = np.asarray(inputs["ln2_g"], np.float32)
    ln2_b = np.asarray(inputs["ln2_b"], np.float32)
    tok_w1 = np.asarray(inputs["tok_w1"], np.float32)
    tok_b1 = np.asarray(inputs["tok_b1"], np.float32)
    tok_w2 = np.asarray(inputs["tok_w2"], np.float32)
    ch_w1 = np.asarray(inputs["ch_w1"], np.float32)
    ch_b1 = np.asarray(inputs["ch_b1"], np.float32)
    ch_w2 = np.asarray(inputs["ch_w2"], np.float32)
    ch_b2 = np.asarray(inputs["ch_b2"], np.float32)

    BF = ml_dtypes.bfloat16
    w1c = np.cumsum(tok_w1, axis=0, dtype=np.float64).astype(np.float32)
    colsum1 = w1c.sum(axis=0, dtype=np.float64).astype(np.float32)
    bias1 = ln1_b[:, None] * colsum1[None, :] + tok_b1[None, :]
    cw1 = (ln2_g[:, None] * ch_w1).astype(np.float32)
    cb1 = (ch_b1 + ch_w1.T @ ln2_b).astype(np.float32)

    nontrivial_bias1 = bool(np.any(bias1 != 0.0) or np.any(cb1 != 0.0))
    nontrivial_cb2 = bool(np.any(ch_b2 != 0.0))

    shared = {
        "w1c": w1c.astype(BF),
        "w2": np.ascontiguousarray(tok_w2).astype(BF),
        "g1": ln1_g.reshape(P, 1).copy(),
        "bias1": np.ascontiguousarray(bias1, np.float32),
        "cw1": cw1.astype(BF),
        "cb1": np.ascontiguousarray(cb1.reshape(NCI, P).T.copy()),
        "cw2": np.ascontiguousarray(ch_w2).astype(BF),
        "cb2": ch_b2.reshape(P, 1).astype(np.float32).copy(),
        "onesc": np.ones((P, 1), BF),
    }
    return x.astype(BF), shared, nontrivial_bias1, nontrivial_cb2


def kernel(**inputs) -> np.ndarray:
    from concourse.bass_utils import run_bass_kernel_spmd

    x, shared, nb1, nb2 = _host_prep(inputs)

    key = (nb1, nb2)
    if key not in _cached:
        _cached[key] = _build(nb1, nb2)
    nc = _cached[key]

    in_maps = []
    for c in range(NCORES):
        m = dict(shared)
        m["x"] = np.ascontiguousarray(x[c * BL:(c + 1) * BL])
        in_maps.append(m)

    res = run_bass_kernel_spmd(nc, in_maps, core_ids=list(range(NCORES)))
    out = np.concatenate(
        [r["out"].transpose(0, 2, 1) for r in res.results], axis=0)
    return np.ascontiguousarray(out, dtype=np.float32)


if __name__ == "__main__":
    rng = np.random.default_rng(0)
    ins = {
        "x": rng.standard_normal((B, N, H)).astype(np.float32),
        "ln1_g": np.ones(H, np.float32),
        "ln1_b": np.zeros(H, np.float32),
        "ln2_g": np.ones(H, np.float32),
        "ln2_b": np.zeros(H, np.float32),
        "tok_w1": (rng.standard_normal((N, TM)) * 0.02).astype(np.float32),
        "tok_b1": np.zeros(TM, np.float32),
        "tok_w2": (rng.standard_normal((TM, N)) * 0.02).astype(np.float32),
        "tok_b2": np.zeros(N, np.float32),
        "ch_w1": (rng.standard_normal((H, CM)) * 0.02).astype(np.float32),
        "ch_b1": np.zeros(CM, np.float32),
        "ch_w2": (rng.standard_normal((CM, H)) * 0.02).astype(np.float32),
        "ch_b2": np.zeros(H, np.float32),
    }
    out = kernel(**ins)
    print("out", out.shape, out.dtype)


# revision 22
# speedup vs baseline: 1.4562x; 1.0244x over previous
"""Trainium2 Bass kernel for nn_AutoregressiveMixerBlock.

Reference computation (per batch b):
  y  = LN_H(x)                                    # layer norm over H
  t  = revcumsum_N(y)                             # t[j] = sum_{i>=j} y[i]
  h  = gelu(t^T @ tok_w1 + tok_b1)                # [H, TM]
  y2 = (h @ tok_w2 + tok_b2)^T                    # [N, H]
  y3 = LN_H(y2)
  out = gelu(y3 @ ch_w1 + ch_b1) @ ch_w2 + ch_b2  # [N, H]

Algebraic folds (exact in real arithmetic, applied on host):
  * revcumsum+matmul:  sum_j t[j,h] w1[j,m] = sum_i y[i,h] W1c[i,m]
    with W1c = cumsum(tok_w1, axis=0) -> no on-device cumsum at all.
  * LN1 gain/bias move past the token matmul.
  * tok_b2 and the LN2 mean both vanish by centering h^T by its
    per-row (over H) mean before the second token matmul.
  * LN2 gain/bias fold into ch_w1 / ch_b1.

Device strategy (per core, 2 batches):
  * all matmul operands bf16 (PE 1 cyc/row), f32 PSUM accumulate.
  * LN1 stats WITHOUT vector-engine reductions: a DMA-transposed copy
    of x ([H, N] layout) is squared on the scalar engine, and both
    sum_h and sum_h^2 per token come from tiny ones-column matmuls
    (contraction over partitions = H) into compact psum columns.
  * rsqrt everywhere via the fast-inverse-sqrt int hack (int ops on
    DVE, Newton multiplies on GPSIMD) -> the scalar engine only ever
    runs Gelu/Square/Copy/Identity = ONE activation table set.
  * LN2 variance reduced the same way (ones-column matmuls on the
    squared token-mix output); the per-token rstd is broadcast to a
    [128, N] bf16 tile via a DRAM roundtrip (store / transposed load /
    store / stride-0 broadcast load).
  * channel MLP output accumulates into bank 0 of the gelu-input psum
    tile (already consumed) and the two batches' channel-MLP sweeps
    run back-to-back with two alternating 4-bank psum tiles.
"""

import numpy as np

B, N, H = 16, 8192, 128
TM, CM = 256, 512
EPS = 1e-5
NCORES = 8
BL = B // NCORES          # batches per core
P = 128                   # partitions
NC_TOK = N // P           # 64 token chunks of 128
NJ = 16                   # j chunks per batch
JW = N // NJ              # 512 tokens per j chunk
TPJ = JW // P             # 4 token chunks per j chunk
KTM = TM // P             # 2 k-chunks for the second token matmul
NCI = CM // P             # 4 chunks of the channel hidden dim
MAGIC1 = 0x5F3759DF + 1   # fast-rsqrt magic (for the xor/asr variant)

_cached = {}


def _build(nontrivial_bias1, nontrivial_cb2):
    import contextlib

    import concourse.mybir as mybir
    import concourse.tile as tile
    from concourse import bacc
    from concourse.masks import make_identity

    F32 = mybir.dt.float32
    BF16 = mybir.dt.bfloat16
    I32 = mybir.dt.int32
    AF = mybir.ActivationFunctionType
    ALU = mybir.AluOpType

    nc = bacc.Bacc()

    # ---- DRAM tensors -------------------------------------------------
    x_d = nc.dram_tensor("x", [BL, N, H], BF16, kind="ExternalInput")
    w1c_d = nc.dram_tensor("w1c", [N, TM], BF16, kind="ExternalInput")
    w2_d = nc.dram_tensor("w2", [TM, N], BF16, kind="ExternalInput")
    g1_d = nc.dram_tensor("g1", [P, 1], F32, kind="ExternalInput")
    bias1_d = nc.dram_tensor("bias1", [P, TM], F32, kind="ExternalInput")
    cw1_d = nc.dram_tensor("cw1", [H, CM], BF16, kind="ExternalInput")
    cb1_d = nc.dram_tensor("cb1", [P, NCI], F32, kind="ExternalInput")
    cw2_d = nc.dram_tensor("cw2", [CM, H], BF16, kind="ExternalInput")
    cb2_d = nc.dram_tensor("cb2", [P, 1], F32, kind="ExternalInput")
    onesc_d = nc.dram_tensor("onesc", [P, 1], BF16, kind="ExternalInput")
    # rstd scratch: [t, c] then [c, t] layouts (DMA-transposed between)
    rs1_d = nc.dram_tensor("rs1", [BL, P, P], BF16, kind="ExternalOutput")
    rs2_d = nc.dram_tensor("rs2", [BL, NC_TOK, P], BF16,
                           kind="ExternalOutput")
    out_d = nc.dram_tensor("out", [BL, H, N], F32, kind="ExternalOutput")

    # DRAM views
    x_v = [x_d[b].rearrange("(c p) h -> p c h", p=P) for b in range(BL)]
    w1c_v = w1c_d[:].rearrange("(c p) m -> p c m", p=P)
    w2_v = w2_d[:].rearrange("(k p) (j n) -> p k j n", p=P, n=JW)
    cw2_v = cw2_d[:].rearrange("(ci p) h -> p ci h", p=P)
    out_v = [out_d[b] for b in range(BL)]

    with tile.TileContext(nc) as tc:
        with contextlib.ExitStack() as ctx:
            const = ctx.enter_context(tc.tile_pool(name="const", bufs=1))
            big = ctx.enter_context(tc.tile_pool(name="big", bufs=1))
            stat = ctx.enter_context(tc.tile_pool(name="stat", bufs=1))
            small = ctx.enter_context(tc.tile_pool(name="small", bufs=4))
            w2s = ctx.enter_context(tc.tile_pool(name="w2s", bufs=6))
            sqp = ctx.enter_context(tc.tile_pool(name="sqp", bufs=3))
            ynp = ctx.enter_context(tc.tile_pool(name="ynp", bufs=3))
            g2p = ctx.enter_context(tc.tile_pool(name="g2p", bufs=2))
            outp = ctx.enter_context(tc.tile_pool(name="outp", bufs=3))

            # big retained tiles; bcast[b] doubles as the transposed-x
            # scratch and y2r[b] as the squared-x scratch during LN1
            # (both are otherwise unused until phase 3)
            y2r = [big.tile([P, N], BF16, name=f"y2r{b}")
                   for b in range(BL)]
            bcast = [big.tile([P, N], BF16, name=f"bc{b}")
                     for b in range(BL)]
            xT_sb = bcast

            # ---- input DMAs (transposed-x copies first: they gate the
            # LN1 stats which gate everything) ---------------------------
            x_sb = [big.tile([P, NC_TOK, H], BF16, name=f"x{b}")
                    for b in range(BL)]
            nc.sync.dma_start(x_sb[0], x_v[0])
            nc.sync.dma_start(xT_sb[0], x_d[0], transpose=True)
            nc.sync.dma_start(xT_sb[1], x_d[1], transpose=True)
            nc.sync.dma_start(x_sb[1], x_v[1])
            g1_sb = const.tile([P, 1], F32)
            nc.sync.dma_start(g1_sb, g1_d[:])
            cw1_sb = const.tile([H, CM], BF16)
            nc.sync.dma_start(cw1_sb, cw1_d[:])
            cb1_sb = const.tile([P, NCI], F32)
            nc.sync.dma_start(cb1_sb, cb1_d[:])
            cw2_sb = const.tile([P, NCI, H], BF16)
            nc.sync.dma_start(cw2_sb, cw2_v)
            onesc_sb = const.tile([P, 1], BF16)
            nc.sync.dma_start(onesc_sb, onesc_d[:])
            ident = const.tile([P, P], BF16)
            make_identity(nc, ident)
            if nontrivial_bias1:
                bias1_sb = const.tile([P, TM], F32)
                nc.sync.dma_start(bias1_sb, bias1_d[:])
            if nontrivial_cb2:
                cb2_sb = const.tile([P, 1], F32)
                nc.sync.dma_start(cb2_sb, cb2_d[:])
            w1c_sb = big.tile([P, NC_TOK, TM], BF16, name="w1c")
            nc.sync.dma_start(w1c_sb, w1c_v)
            g1_t = small.tile([P, 1], F32)
            nc.vector.tensor_copy(g1_t, g1_sb)

            # ---------------------------------------------------------
            def rsqrt_chain(dst, vsrc, tmp_a, tmp_b):
                """dst = 1/sqrt(vsrc) (vsrc f32 SBUF [P, n]).

                Fast-inverse-sqrt seed (int ops on the DVE's fp32-ALU
                int path, accurate to ~1e-6 here) + 2 Newton steps, all
                on DVE: the tiles are tiny so cross-engine handoff
                latency would dominate any offload. tmp_a/tmp_b f32 same
                shape; dst may be bf16.
                """
                iv = vsrc[:].bitcast(I32)
                nc.vector.tensor_scalar(
                    out=tmp_a[:].bitcast(I32), in0=iv, scalar1=-1, scalar2=1,
                    op0=ALU.bitwise_xor, op1=ALU.arith_shift_right)
                nc.vector.tensor_scalar(
                    out=tmp_b[:].bitcast(I32), in0=tmp_a[:].bitcast(I32),
                    scalar1=MAGIC1, scalar2=None, op0=ALU.add)
                for it in range(2):
                    nc.vector.tensor_tensor(tmp_a, tmp_b, tmp_b, ALU.mult)
                    nc.vector.tensor_tensor(tmp_a, tmp_a, vsrc, ALU.mult)
                    nc.vector.tensor_scalar(
                        out=tmp_a, in0=tmp_a, scalar1=-0.5, scalar2=1.5,
                        op0=ALU.mult, op1=ALU.add)
                    nc.vector.tensor_tensor(
                        dst if it == 1 else tmp_b, tmp_b, tmp_a, ALU.mult)

            mu1 = []
            rstd1 = []
            nmr1 = []
            for b in range(BL):
                mu1.append(stat.tile([P, NC_TOK], F32, name=f"mu{b}"))
                rstd1.append(stat.tile([P, NC_TOK], F32, name=f"rs{b}"))
                nmr1.append(stat.tile([P, NC_TOK], F32, name=f"nm{b}"))

            with (
                tc.tile_pool(name="pstat", bufs=1, space="PSUM") as pstat,
                tc.tile_pool(name="ps1", bufs=2, space="PSUM") as ps1,
                tc.tile_pool(name="pst", bufs=1, space="PSUM") as pst,
                tc.tile_pool(name="ps2", bufs=2, space="PSUM") as ps2,
                tc.tile_pool(name="vcp", bufs=1, space="PSUM") as vcp,
            ):
                vc = vcp.tile([P, BL * NC_TOK], F32, name="vc")
                h1c = [[None] * KTM for _ in range(BL)]
                sq_tiles = {}

                def emit_stats(b):
                    """LN1 per-token mean/rstd without DVE reductions."""
                    sqT = y2r[b]
                    nc.scalar.activation(sqT, xT_sb[b], AF.Square)
                    ps = pstat.tile([P, P], F32, tag="pstat")
                    for c in range(NC_TOK):
                        nc.tensor.matmul(
                            ps[:, c:c + 1],
                            xT_sb[b][:, c * P:(c + 1) * P], onesc_sb,
                            start=True, stop=True)
                    for c in range(NC_TOK):
                        nc.tensor.matmul(
                            ps[:, NC_TOK + c:NC_TOK + c + 1],
                            sqT[:, c * P:(c + 1) * P], onesc_sb,
                            start=True, stop=True)
                    vs = stat.tile([P, NC_TOK], F32, name=f"vs1_{b}")
                    ta = stat.tile([P, NC_TOK], F32, name=f"ta1_{b}")
                    tb = stat.tile([P, NC_TOK], F32, name=f"tb1_{b}")
                    # mu = sums/H ; var+eps = sumsq/H + EPS - mu^2
                    nc.vector.tensor_scalar_mul(mu1[b], ps[:, 0:NC_TOK],
                                                1.0 / H)
                    nc.vector.tensor_scalar(
                        out=ta, in0=ps[:, NC_TOK:], scalar1=1.0 / H,
                        scalar2=EPS, op0=ALU.mult, op1=ALU.add)
                    nc.vector.tensor_tensor(tb, mu1[b], mu1[b], ALU.mult)
                    nc.vector.tensor_tensor(vs, ta, tb, ALU.subtract)
                    rsqrt_chain(rstd1[b], vs, ta, tb)
                    # -mu*rstd, the per-chunk bias for the ACT xn path
                    nc.vector.tensor_tensor(ta, mu1[b], rstd1[b], ALU.mult)
                    nc.vector.tensor_scalar_mul(nmr1[b], ta, -1.0)

                def emit_mm1(b):
                    psum1 = ps1.tile([P, TM], F32, tag="ps1")
                    for c in range(NC_TOK):
                        xn = small.tile([P, P], BF16, tag="xn")
                        if c % 2 == 0:
                            nc.vector.tensor_scalar(
                                out=xn,
                                in0=x_sb[b][:, c, :],
                                scalar1=mu1[b][:, c:c + 1],
                                scalar2=rstd1[b][:, c:c + 1],
                                op0=ALU.subtract,
                                op1=ALU.mult,
                            )
                        else:
                            nc.scalar.activation(
                                xn, x_sb[b][:, c, :], AF.Identity,
                                bias=nmr1[b][:, c:c + 1],
                                scale=rstd1[b][:, c:c + 1])
                        nc.tensor.matmul(
                            psum1, xn, w1c_sb[:, c, :],
                            start=(c == 0), stop=(c == NC_TOK - 1))
                    return psum1

                def emit_ph2(b, psum1):
                    h1 = small.tile([P, TM], BF16, tag="h1")
                    if nontrivial_bias1:
                        h1f = small.tile([P, TM], F32, tag="h1f")
                        nc.vector.tensor_scalar_mul(h1f, psum1, g1_t)
                        nc.vector.tensor_add(h1f, h1f, bias1_sb)
                        nc.scalar.activation(h1, h1f, AF.Gelu)
                    else:
                        nc.scalar.activation(h1, psum1, AF.Gelu, scale=g1_t)
                    for k in range(KTM):
                        hk = h1[:, k * P:(k + 1) * P]
                        ps_t = pst.tile([P, P], BF16, tag="pst")
                        nc.tensor.transpose(ps_t, hk, ident)
                        hm = pst.tile([P, 1], F32, tag="hm")
                        nc.tensor.matmul(hm, hk, onesc_sb,
                                         start=True, stop=True)
                        nhm = small.tile([P, 1], F32, tag="nhm")
                        nc.scalar.activation(nhm, hm, AF.Copy,
                                             scale=float(-1.0 / H))
                        hc = small.tile([P, P], BF16, tag="h1c",
                                        name=f"hc{b}_{k}")
                        nc.scalar.activation(hc, ps_t, AF.Identity,
                                             bias=nhm)
                        h1c[b][k] = hc

                def emit_3a_mm(b, j, y2r_engine, sq_engine):
                    """token matmul 2 for (b, j) + psum drain + square."""
                    w2t = []
                    for k in range(KTM):
                        wt = w2s.tile([P, JW], BF16, tag="w2")
                        nc.sync.dma_start(wt, w2_v[:, k, j, :])
                        w2t.append(wt)
                    p2 = ps2.tile([P, JW], F32, tag="ps2")
                    for k in range(KTM):
                        nc.tensor.matmul(
                            p2, h1c[b][k], w2t[k],
                            start=(k == 0), stop=(k == KTM - 1))
                    yj = y2r[b][:, j * JW:(j + 1) * JW]
                    if y2r_engine == "act":
                        nc.scalar.activation(yj, p2, AF.Copy)
                    else:
                        nc.vector.tensor_copy(yj, p2)
                    sq = sqp.tile([P, JW], BF16, tag="sq")
                    if sq_engine == "gpsimd":
                        nc.gpsimd.tensor_tensor(sq, yj, yj, ALU.mult)
                    else:
                        nc.vector.tensor_tensor(sq, yj, yj, ALU.mult)
                    sq_tiles[(b, j)] = sq

                def emit_3a_vc(b, j):
                    """compact per-token variance columns for (b, j)."""
                    sq = sq_tiles.pop((b, j))
                    for c in range(TPJ):
                        col = b * NC_TOK + j * TPJ + c
                        nc.tensor.matmul(
                            vc[:, col:col + 1],
                            sq[:, c * P:(c + 1) * P], onesc_sb,
                            start=True, stop=True)

                def emit_chain2(b):
                    vs = stat.tile([P, NC_TOK], F32, name=f"vs2_{b}")
                    ta = stat.tile([P, NC_TOK], F32, name=f"ta2_{b}")
                    tb = stat.tile([P, NC_TOK], F32, name=f"tb2_{b}")
                    # padded to [P, P]: the XBAR transpose DMA needs a
                    # multiple-of-128 free dim
                    rc = stat.tile([P, P], BF16, name=f"rc2_{b}")
                    nc.gpsimd.memset(rc[:, NC_TOK:], 0.0)
                    nc.vector.tensor_scalar(
                        out=vs, in0=vc[:, b * NC_TOK:(b + 1) * NC_TOK],
                        scalar1=1.0 / H, scalar2=EPS,
                        op0=ALU.mult, op1=ALU.add)
                    rsqrt_chain(rc[:, 0:NC_TOK], vs, ta, tb)
                    # rc[t, c] --plain--> rs1 --transposed--> rr[c, t]
                    #   --plain--> rs2 --stride-0 broadcast--> bcast[p, c*t]
                    nc.sync.dma_start(rs1_d[b], rc)
                    rr = small.tile([P, P], BF16, tag="rr")
                    nc.sync.dma_start(rr, rs1_d[b], transpose=True)
                    nc.sync.dma_start(rs2_d[b], rr[0:NC_TOK, :])
                    nc.sync.dma_start(
                        bcast[b],
                        rs2_d[b].rearrange(
                            "c t -> (c t)").partition_broadcast(P))

                def emit_3b(b, j, psr):
                    yn = ynp.tile([P, JW], BF16, tag="yn")
                    nc.vector.tensor_tensor(
                        yn, y2r[b][:, j * JW:(j + 1) * JW],
                        bcast[b][:, j * JW:(j + 1) * JW], ALU.mult)
                    raw = psr.tile([P, NCI * JW], F32, tag="psr")
                    for ci in range(NCI):
                        nc.tensor.matmul(
                            raw[:, ci * JW:(ci + 1) * JW],
                            cw1_sb[:, ci * P:(ci + 1) * P],
                            yn, start=True, stop=True)
                    g2 = g2p.tile([P, NCI * JW], BF16, tag="g2")
                    if nontrivial_bias1:
                        for ci in range(NCI):
                            nc.scalar.activation(
                                g2[:, ci * JW:(ci + 1) * JW],
                                raw[:, ci * JW:(ci + 1) * JW],
                                AF.Gelu, bias=cb1_sb[:, ci:ci + 1])
                    else:
                        nc.scalar.activation(g2, raw, AF.Gelu)
                    # accumulate the output into bank 0 of `raw` (fully
                    # consumed by the gelu above) to stay in 8 banks
                    po = raw[:, 0:JW]
                    for ci in range(NCI):
                        nc.tensor.matmul(
                            po, cw2_sb[:, ci, :],
                            g2[:, ci * JW:(ci + 1) * JW],
                            start=(ci == 0), stop=(ci == NCI - 1))
                    osb = outp.tile([P, JW], F32, tag="osb")
                    if nontrivial_cb2:
                        nc.vector.tensor_scalar(
                            out=osb, in0=po, scalar1=cb2_sb,
                            scalar2=None, op0=ALU.add)
                    else:
                        nc.vector.tensor_copy(osb, po)
                    nc.sync.dma_start(out_v[b][:, j * JW:(j + 1) * JW], osb)

                # ---- emission schedule --------------------------------
                emit_stats(0)
                p1_0 = emit_mm1(0)
                emit_ph2(0, p1_0)
                emit_stats(1)
                for j in range(NJ):
                    emit_3a_mm(0, j, "act", "gpsimd")
                    if j > 0:
                        emit_3a_vc(0, j - 1)
                emit_3a_vc(0, NJ - 1)
                emit_chain2(0)
                p1_1 = emit_mm1(1)
                emit_ph2(1, p1_1)
                for j in range(NJ):
                    emit_3a_mm(1, j, "act", "dve")
                    if j > 0:
                        emit_3a_vc(1, j - 1)
                emit_3a_vc(1, NJ - 1)
                emit_chain2(1)

            # channel MLP for both batches, double-buffered 4-bank psum
            with (
                tc.tile_pool(name="psrA", bufs=1, space="PSUM") as psrA,
                tc.tile_pool(name="psrB", bufs=1, space="PSUM") as psrB,
            ):
                for b in range(BL):
                    for j in range(NJ):
                        emit_3b(b, j, psrA if (b * NJ + j) % 2 else psrB)

    nc.compile()
    return nc


def _host_prep(inputs):
    import ml_dtypes

    x = np.asarray(inputs["x"], dtype=np.float32)
    ln1_g = np.asarray(inputs["ln1_g"], np.float32)
    ln1_b = np.asarray(inputs["ln1_b"], np.float32)
    ln2_g = np.asarray(inputs["ln2_g"], np.float32)
    ln2_b = np.asarray(inputs["ln2_b"], np.float32)
    tok_w1 = np.asarray(inputs["tok_w1"], np.float32)
    tok_b1 = np.asarray(inputs["tok_b1"], np.float32)
    tok_w2 = np.asarray(inputs["tok_w2"], np.float32)
    ch_w1 = np.asarray(inputs["ch_w1"], np.float32)
    ch_b1 = np.asarray(inputs["ch_b1"], np.float32)
    ch_w2 = np.asarray(inputs["ch_w2"], np.float32)
    ch_b2 = np.asarray(inputs["ch_b2"], np.float32)

    BF = ml_dtypes.bfloat16
    w1c = np.cumsum(tok_w1, axis=0, dtype=np.float64).astype(np.float32)
    colsum1 = w1c.sum(axis=0, dtype=np.float64).astype(np.float32)
    bias1 = ln1_b[:, None] * colsum1[None, :] + tok_b1[None, :]
    cw1 = (ln2_g[:, None] * ch_w1).astype(np.float32)
    cb1 = (ch_b1 + ch_w1.T @ ln2_b).astype(np.float32)

    nontrivial_bias1 = bool(np.any(bias1 != 0.0) or np.any(cb1 != 0.0))
    nontrivial_cb2 = bool(np.any(ch_b2 != 0.0))

    shared = {
        "w1c": w1c.astype(BF),
        "w2": np.ascontiguousarray(tok_w2).astype(BF),
        "g1": ln1_g.reshape(P, 1).copy(),
        "bias1": np.ascontiguousarray(bias1, np.float32),
        "cw1": cw1.astype(BF),
        "cb1": np.ascontiguousarray(cb1.reshape(NCI, P).T.copy()),
        "cw2": np.ascontiguousarray(ch_w2).astype(BF),
        "cb2": ch_b2.reshape(P, 1).astype(np.float32).copy(),
        "onesc": np.ones((P, 1), BF),
    }
    return x.astype(BF), shared, nontrivial_bias1, nontrivial_cb2


def kernel(**inputs) -> np.ndarray:
    from concourse.bass_utils import run_bass_kernel_spmd

    x, shared, nb1, nb2 = _host_prep(inputs)

    key = (nb1, nb2)
    if key not in _cached:
        _cached[key] = _build(nb1, nb2)
    nc = _cached[key]

    in_maps = []
    for c in range(NCORES):
        m = dict(shared)
        m["x"] = np.ascontiguousarray(x[c * BL:(c + 1) * BL])
        in_maps.append(m)

    res = run_bass_kernel_spmd(nc, in_maps, core_ids=list(range(NCORES)))
    out = np.concatenate(
        [r["out"].transpose(0, 2, 1) for r in res.results], axis=0)
    return np.ascontiguousarray(out, dtype=np.float32)


if __name__ == "__main__":
    rng = np.random.default_rng(0)
    ins = {
        "x": rng.standard_normal((B, N, H)).astype(np.float32),
        "ln1_g": np.ones(H, np.float32),
        "ln1_b": np.zeros(H, np.float32),
        "ln2_g": np.ones(H, np.float32),
        "ln2_b": np.zeros(H, np.float32),
        "tok_w1": (rng.standard_normal((N, TM)) * 0.02).astype(np.float32),
        "tok_b1": np.zeros(TM, np.float32),
        "tok_w2": (rng.standard_normal((TM, N)) * 0.02).astype(np.float32),
        "tok_b2": np.zeros(N, np.float32),
        "ch_w1": (rng.standard_normal((H, CM)) * 0.02).astype(np.float32),
        "ch_b1": np.zeros(CM, np.float32),
        "ch_w2": (rng.standard_normal((CM, H)) * 0.02).astype(np.float32),
        "ch_b2": np.zeros(H, np.float32),
    }
    out = kernel(**ins)
    print("out", out.shape, out.dtype)
